# revision 21
# baseline (speedup 1.0000x reference)
"""Fused sp2norm-MHA kernel for Trainium2, 8 NeuronCores.

Model (per reference):
    qkv = x @ W_qkv.T ; split heads (H=16, hs=64)
    s = (q @ k.T) / sqrt(hs);  w = softplus(s) causal-masked
    out_h = (w @ v) / ||w||_row ;  out = concat(out_h) @ W_proj.T + b_proj

Sharding: core c = (b, g) with b = c // 4 (batch), g = c % 4 (head group of 4).
Each core computes its batch's QKV for its 4 heads, the attention, and a
partial projection over its 256 feature channels. The host sums the 4 partial
projections per batch and adds the bias (the unshard step).

Schedule: single fully-interleaved phase. The attention pipeline (scores ->
softplus(Exp,Ln) -> mask/square -> out/norm matmuls) is ACT-bound (~2.05us
per j-block vs ~1.4us of PE work), so the QKV projection matmuls and the
output projection are DRIP-FED between attention blocks: the PE never idles
long enough for the HAM clock gate to re-throttle it to 1.2 GHz (which cost
the phase-separated version ~90us), and the QKV/proj phases hide entirely
inside the attention's ACT time.

PSUM (8 banks): scores [128,2048] x1 (4) | out.T [128,512] x2 (2) |
norm [128,512] x1 (1) | misc qkv/proj/bcast [128,512] x1 (1).

TWO j-blocks are batched per softplus ACT instruction (the ~293ns fixed
cost per ACTIVATE was ~47us over 160 instrs; batching halves it). In a
batch the narrower diagonal block is computed at the wider block's width;
the extra leading columns fall in the causally-masked region and are
zeroed by a widened [0|triu] mask, so they add exact zeros to the out/norm
accumulations.

The emission is software-pipelined: batch k's out/norm matmuls are emitted
AFTER batch k+1's score matmuls, so the in-order PE queue never stalls on
the ACT->DVE chain of the current batch.

The norm rows for the two head pairs share one PSUM bank (pair0 at
partitions 0/64, pair1 at 32/96) so one Ln+Exp epilogue pair serves all
4 heads of an i-chunk.
"""

import numpy as np
import ml_dtypes
from collections import deque

import concourse.bacc as bacc
import concourse.tile as tile
import concourse.mybir as mybir
from concourse.bass_utils import run_bass_kernel_spmd

# The act-table-set chooser assigns each activation the FIRST set containing
# its function; with the default ordering Exp -> exp_and_others and
# Ln -> natural_log, so alternating Exp/Ln thrashes ACT_TABLE_LOAD (~1.3us
# each, >100 loads). Reorder so the combined Exp+Ln set is preferred.
_orig_get_tables = bacc.get_activation_tables


def _tables_ln_exp_first(arch):
    t = _orig_get_tables(arch)
    key = "natural_log_exp_and_others"
    if key not in t:
        return t
    exp = mybir.ActivationFunctionType.Exp
    ln = mybir.ActivationFunctionType.Ln
    out = {}
    for k, fns in t.items():
        out[k] = fns if k == key else (set(fns) - {exp, ln})
    return out


bacc.get_activation_tables = _tables_ln_exp_first

dt = mybir.dt
F32, F16, BF16 = dt.float32, dt.float16, dt.bfloat16
AF = mybir.ActivationFunctionType

B, T, C, H, HS = 2, 2048, 1024, 16, 64
NCORES = 8
SCALE = 1.0 / np.sqrt(HS)

_CACHE = {}


def _build():
    nc = bacc.Bacc(None, target_bir_lowering=False)

    xT = nc.dram_tensor("xT", [C, T], BF16, kind="ExternalInput")
    wqk = nc.dram_tensor("wqk", [C, 512], BF16, kind="ExternalInput")
    wv = nc.dram_tensor("wv", [C, 256], BF16, kind="ExternalInput")
    wp = nc.dram_tensor("wp", [256, C], BF16, kind="ExternalInput")
    mtri = nc.dram_tensor("mtri", [128, 128], BF16, kind="ExternalInput")
    mtri2 = nc.dram_tensor("mtri2", [128, 256], BF16, kind="ExternalInput")
    out = nc.dram_tensor("out", [T, C], F32, kind="ExternalOutput")

    with tile.TileContext(nc) as tc:
        with (
            tc.tile_pool(name="cst", bufs=1) as cst,
            tc.tile_pool(name="data", bufs=1) as data,
            tc.tile_pool(name="ps", bufs=1, space="PSUM") as ps,
            tc.tile_pool(name="we", bufs=2) as we,
            tc.tile_pool(name="epi", bufs=2) as epi,
            tc.tile_pool(name="outp", bufs=3) as outp,
        ):
            # ---- constants / weights ----
            # Input DMAs are spread across the three DMA-capable engines
            # (sync, scalar, gpsimd): DMA_DIRECT2D occupies its issuing
            # engine for ~0.6us per 128KB, so one queue serializes the
            # ~6MB of inputs into a ~26us lead-in. Three queues + cb/t-quarter
            # splitting get the first QKV group's inputs in ~3us.
            mtri_sb = cst.tile([128, 128], BF16)
            nc.gpsimd.dma_start(mtri_sb, mtri[:])
            mtri2_sb = cst.tile([128, 256], BF16)
            nc.gpsimd.dma_start(mtri2_sb, mtri2[:])
            ones_n = cst.tile([128, 1], BF16)
            nc.vector.memset(ones_n, 1.0)
            ones_b = cst.tile([128, 64], BF16)
            nc.vector.memset(ones_b, 1.0)

            wqk_sb = cst.tile([128, 8, 512], BF16)
            wv_sb = cst.tile([128, 8, 256], BF16)
            xT_sb = data.tile([128, 8, 2048], BF16)
            wqk_r = wqk[:].rearrange("(po pi) j -> pi po j", pi=128)
            wv_r = wv[:].rearrange("(po pi) j -> pi po j", pi=128)
            xT_r = xT[:].rearrange("(po pi) t -> pi po t", pi=128)
            for cb in range(8):
                nc.scalar.dma_start(wqk_sb[:, cb, :], wqk_r[:, cb, :])
            # first QKV groups only need t[0:512]: stream xT by t-quarter
            for cb in range(8):
                eng = nc.sync if cb % 2 == 0 else nc.gpsimd
                eng.dma_start(xT_sb[:, cb, 0:512], xT_r[:, cb, 0:512])
            for cb in range(8):
                nc.gpsimd.dma_start(wv_sb[:, cb, :], wv_r[:, cb, :])
            for tq in range(1, 4):
                for cb in range(8):
                    eng = nc.sync if cb % 2 == 0 else nc.gpsimd
                    eng.dma_start(xT_sb[:, cb, tq * 512:(tq + 1) * 512],
                                  xT_r[:, cb, tq * 512:(tq + 1) * 512])
            wp_sb = cst.tile([128, 2, 1024], BF16)
            nc.scalar.dma_start(wp_sb, wp[:].rearrange("(po pi) e -> pi po e", pi=128))

            # qkT: block 0,1 = q head-pairs; block 2,3 = k head-pairs.
            # Partition rows (h%2)*64..+64 inside each block = one head.
            qkT = data.tile([128, 4, 2048], BF16)
            v_sb = data.tile([128, 16, 256], BF16)
            yT = data.tile([128, 2, 2048], BF16)

            # ---------- drip-fed work items (QKV groups, proj chunks) ----------
            # Each item is ~0.9-1.8us of PE work, emitted between attention
            # blocks so the in-order PE queue always has ready work.
            def emit_qk_group(jb, tcc):
                # qkT[:, jb, tcc*512:+512] = wqk_blk.T @ xT_chunk
                pq = ps.tile([128, 512], F32, tag="m", bufs=1)
                for cb in range(8):
                    nc.tensor.matmul(
                        pq,
                        wqk_sb[:, cb, jb * 128:(jb + 1) * 128],
                        xT_sb[:, cb, tcc * 512:(tcc + 1) * 512],
                        start=(cb == 0), stop=(cb == 7),
                    )
                nc.vector.tensor_copy(qkT[:, jb, tcc * 512:(tcc + 1) * 512], pq)

            def emit_v_group(tb):
                pv = ps.tile([128, 512], F32, tag="m", bufs=1)
                for cb in range(8):
                    nc.tensor.matmul(
                        pv[:, 0:256],
                        xT_sb[:, cb, tb * 128:(tb + 1) * 128],
                        wv_sb[:, cb, :],
                        start=(cb == 0), stop=(cb == 7),
                    )
                nc.vector.tensor_copy(v_sb[:, tb, :], pv[:, 0:256])

            def emit_proj(tcc, nk, sbank=False):
                if sbank:
                    pp_s = ps.tile([128, 2048], F32, tag="s", bufs=1, name="pp_s")
                    pp = pp_s[:, 0:512]
                else:
                    pp = ps.tile([128, 512], F32, tag="m", bufs=1)
                for kb in range(2):
                    nc.tensor.matmul(
                        pp,
                        yT[:, kb, tcc * 128:(tcc + 1) * 128],
                        wp_sb[:, kb, nk * 512:(nk + 1) * 512],
                        start=(kb == 0), stop=(kb == 1),
                    )
                os_ = outp.tile([128, 512], F32, tag="os")
                nc.vector.tensor_copy(os_, pp)
                eng = nc.scalar if (tcc + nk) % 2 else nc.sync
                eng.dma_start(
                    out[tcc * 128:(tcc + 1) * 128, nk * 512:(nk + 1) * 512], os_)

            qkv_q = deque()
            proj_q = deque()

            def drip(k):
                for _ in range(k):
                    if qkv_q:
                        qkv_q.popleft()()
                    elif proj_q:
                        t, n = proj_q.popleft()
                        emit_proj(t, n)
                    else:
                        return

            # ---------- attention block ----------
            PN_ROW = {0: (0, 64), 1: (32, 96)}   # pn partition rows per pair

            # pending[0] holds the deferred out/norm emission of the previous
            # block: it is emitted AFTER the next block's score matmuls so
            # those reach the in-order PE queue first and fill the Exp shadow.
            pending = [None]

            def flush_pending():
                if pending[0] is not None:
                    pending[0]()
                    pending[0] = None

            def emit_att_batch(hp, ic, bb, po, pn):
                # batch = j-blocks (2bb, 2bb+1); njb = 4ic+4 is always even.
                njb = 4 * ic + 4
                jb0, jb1 = 2 * bb, 2 * bb + 1
                m0 = jb0 - 4 * ic
                # both blocks computed at the wider block's width N
                N = 512 if m0 < 0 else 512 - 128 * m0
                diag = m0 >= 0
                ioff = ic * 512 + (512 - N)
                qblk, kblk = hp, 2 + hp
                ps_ = ps.tile([128, 2048], F32, tag="s", bufs=1)
                # scores (transposed): K=64, heads row-packed; segments:
                # [jb0/headA | jb0/headB | jb1/headA | jb1/headB] at 512 pitch
                for si, (jb, p0, p1) in enumerate(
                        ((jb0, 0, 64), (jb0, 64, 128),
                         (jb1, 0, 64), (jb1, 64, 128))):
                    nc.tensor.matmul(
                        ps_[:, si * 512:si * 512 + N],
                        qkT[p0:p1, kblk, jb * 128:(jb + 1) * 128],
                        qkT[p0:p1, qblk, ioff:ioff + N],
                        start=True, stop=True,
                    )
                # softplus = Ln(Exp(s/8) + 1), fp16 intermediate, one ACT
                # instruction per pass over all 4 segments.
                e = we.tile([128, 2048], F16, tag="e")
                w = we.tile([128, 2048], BF16, tag="w", bufs=3)
                w2 = we.tile([128, 2048], BF16, tag="w2", bufs=3)
                if N == 512:
                    nc.scalar.activation(e, ps_, AF.Exp, scale=SCALE)
                    nc.scalar.activation(w, e, AF.Ln, bias=1.0)
                else:
                    ps4 = ps_.rearrange("p (b n) -> p b n", b=4)[:, :, 0:N]
                    e4 = e.rearrange("p (b n) -> p b n", b=4)[:, :, 0:N]
                    w4 = w.rearrange("p (b n) -> p b n", b=4)[:, :, 0:N]
                    nc.scalar.activation(e4, ps4, AF.Exp, scale=SCALE)
                    nc.scalar.activation(w4, e4, AF.Ln, bias=1.0)
                if diag:
                    # jb0: diagonal sub-block = leading 128 cols of each head
                    # segment; jb1: leading 128 are fully masked (they were
                    # computed only to keep the batch width uniform) and the
                    # next 128 are its diagonal -> [0|triu] 256-wide mask.
                    nc.vector.tensor_mul(w[:, 0:128], w[:, 0:128], mtri_sb)
                    nc.vector.tensor_mul(w[:, 512:640], w[:, 512:640], mtri_sb)
                    nc.vector.tensor_mul(w[:, 1024:1280], w[:, 1024:1280],
                                         mtri2_sb)
                    nc.vector.tensor_mul(w[:, 1536:1792], w[:, 1536:1792],
                                         mtri2_sb)
                if N == 512:
                    nc.vector.tensor_mul(w2, w, w)
                else:
                    w4 = w.rearrange("p (b n) -> p b n", b=4)[:, :, 0:N]
                    w24 = w2.rearrange("p (b n) -> p b n", b=4)[:, :, 0:N]
                    nc.vector.tensor_mul(w24, w4, w4)

                # out/norm of the PREVIOUS batch go to the PE queue now...
                flush_pending()
                drip(1)

                def out_norm(first=(bb == 0), last=(jb1 == njb - 1), jb0=jb0,
                             jb1=jb1, hp=hp, N=N, w=w, w2=w2, po=po, pn=pn):
                    hA, hB = 2 * hp, 2 * hp + 1
                    rA, rB = PN_ROW[hp]
                    for bi, jb in ((0, jb0), (1, jb1)):
                        st = first and bi == 0
                        sp = last and bi == 1
                        nc.tensor.matmul(
                            po[0:64, 512 - N:512],
                            v_sb[:, jb, hA * 64:hA * 64 + 64],
                            w[:, bi * 1024:bi * 1024 + N],
                            start=st, stop=sp, tile_position=(0, 0),
                        )
                        nc.tensor.matmul(
                            po[64:128, 512 - N:512],
                            v_sb[:, jb, hB * 64:hB * 64 + 64],
                            w[:, bi * 1024 + 512:bi * 1024 + 512 + N],
                            start=st, stop=sp, tile_position=(0, 64),
                        )
                        nc.tensor.matmul(
                            pn[rA:rA + 1, 512 - N:512], ones_n,
                            w2[:, bi * 1024:bi * 1024 + N],
                            start=st, stop=sp, tile_position=(0, rA),
                        )
                        nc.tensor.matmul(
                            pn[rB:rB + 1, 512 - N:512], ones_n,
                            w2[:, bi * 1024 + 512:bi * 1024 + 512 + N],
                            start=st, stop=sp, tile_position=(0, rB),
                        )

                # ...this batch's out/norm wait for the next batch's scores
                pending[0] = out_norm

            # ---------- main interleaved schedule ----------
            # PE warmup: ~3.5us of back-to-back tiny matmuls trips the HAM
            # activity window so the clock gate opens (1.2 -> 2.4 GHz) before
            # the first real QKV matmuls; they only depend on the ones memset.
            wrm = cst.tile([128, 512], BF16)
            nc.vector.memset(wrm, 0.0)
            warm_ps = ps.tile([128, 2048], F32, tag="s", bufs=1, name="warm")
            for _ in range(10):
                nc.tensor.matmul(warm_ps[0:64, 0:512], ones_b, wrm,
                                 start=True, stop=True)
            # prereq: q-pair0/k-pair0 i-chunk 0, v block 0
            emit_qk_group(0, 0)
            emit_qk_group(2, 0)
            emit_v_group(0)
            # Release QKV groups just-in-time (keyed by (ic, hp) attention
            # segment) so PE drip work stays dense through the LATE i-chunks,
            # where the attention itself leaves the PE half idle -- otherwise
            # the HAM clock gate re-throttles the PE to 1.2 GHz there.
            RELEASES = {
                (0, 0): [("v", 1), ("v", 2), ("v", 3),
                         ("qk", 1, 0), ("qk", 3, 0)],
                (0, 1): [("qk", 0, 1), ("qk", 2, 1),
                         ("v", 4), ("v", 5), ("v", 6), ("v", 7)],
                (1, 0): [("qk", 1, 1), ("qk", 3, 1)],
                (1, 1): [("qk", 0, 2), ("qk", 2, 2),
                         ("v", 8), ("v", 9), ("v", 10), ("v", 11)],
                (2, 0): [("qk", 1, 2), ("qk", 3, 2)],
                (2, 1): [("qk", 0, 3), ("qk", 2, 3)],
                (3, 0): [("v", 12), ("v", 13), ("v", 14), ("v", 15),
                         ("qk", 1, 3), ("qk", 3, 3)],
            }

            def release(ic, hp):
                for it in RELEASES.get((ic, hp), []):
                    if it[0] == "v":
                        qkv_q.append(lambda tb=it[1]: emit_v_group(tb))
                    else:
                        qkv_q.append(
                            lambda jb=it[1], tcc=it[2]: emit_qk_group(jb, tcc))

            # The chunk epilogue is ALSO deferred: it is emitted after the
            # NEXT chunk's first batch so its ACT ops (Ln/Exp on the norms)
            # slot in behind that batch's softplus instead of stalling the
            # ACT queue at the chunk boundary.
            pending_epi = [None]

            def flush_epi():
                if pending_epi[0] is not None:
                    pending_epi[0]()
                    pending_epi[0] = None

            def make_epi(ic, pn, po_hp):
                def epi_fn():
                    # ---- chunk epilogue: y = out.T * rsqrt(norm2), both
                    # pairs. rsqrt = Exp(-0.5*Ln(x)): stays in the Exp/Ln set.
                    nrm = epi.tile([128, 512], F32, tag="nrm")
                    nc.scalar.activation(nrm, pn, AF.Ln)
                    rs = epi.tile([128, 512], BF16, tag="rs")
                    nc.scalar.activation(rs, nrm, AF.Exp, scale=-0.5)
                    # keep the PE fed through the epilogue's ACT/DVE window
                    drip(2)
                    for hp in range(2):
                        rA, rB = PN_ROW[hp]
                        pb = ps.tile([128, 512], F32, tag="m", bufs=1)
                        nc.tensor.matmul(pb[0:64, :], ones_b[rA:rA + 1, :],
                                         rs[rA:rA + 1, :], start=True,
                                         stop=True, tile_position=(rA, 0))
                        nc.tensor.matmul(pb[64:128, :], ones_b[rB:rB + 1, :],
                                         rs[rB:rB + 1, :], start=True,
                                         stop=True, tile_position=(rB, 64))
                        rb = epi.tile([128, 512], F32, tag="rb")
                        nc.vector.tensor_copy(rb, pb)
                        nc.vector.tensor_mul(
                            yT[:, hp, ic * 512:(ic + 1) * 512], po_hp[hp], rb)
                    # projection for this i-chunk now has all its inputs
                    for tcc in range(ic * 4, ic * 4 + 4):
                        for nk in range(2):
                            proj_q.append((tcc, nk))
                return epi_fn

            for ic in range(4):
                pn = ps.tile([128, 512], F32, tag="n", bufs=1)
                po_hp = {}
                for hp in range(2):
                    release(ic, hp)
                    po = ps.tile([128, 512], F32, tag="o", bufs=2)
                    po_hp[hp] = po
                    for bb in range(2 * ic + 2):
                        emit_att_batch(hp, ic, bb, po, pn)
                        if hp == 0 and bb == 0:
                            flush_epi()   # previous chunk's epilogue
                        if ic == 0:
                            drip(2)   # extra drip early: QKV is front-loaded
                flush_pending()
                pending_epi[0] = make_epi(ic, pn, po_hp)
            flush_epi()

            # drain whatever is left (tail projection chunks): the four
            # score banks are free now -- run 4 chunks in parallel through
            # the quarters of one [128,2048] allocation, DMAs spread over
            # 4 engine queues.
            while qkv_q:
                qkv_q.popleft()()
            tail = list(proj_q)
            proj_q.clear()
            engs = [nc.sync, nc.scalar, nc.gpsimd]
            for i in range(0, len(tail), 4):
                pp_s = ps.tile([128, 2048], F32, tag="s", bufs=1, name="pp_s")
                for qi, (t, n) in enumerate(tail[i:i + 4]):
                    pp = pp_s[:, qi * 512:(qi + 1) * 512]
                    for kb in range(2):
                        nc.tensor.matmul(
                            pp,
                            yT[:, kb, t * 128:(t + 1) * 128],
                            wp_sb[:, kb, n * 512:(n + 1) * 512],
                            start=(kb == 0), stop=(kb == 1),
                        )
                    os_ = outp.tile([128, 512], F32, tag="ost", bufs=4)
                    nc.vector.tensor_copy(os_, pp)
                    engs[qi % 3].dma_start(
                        out[t * 128:(t + 1) * 128, n * 512:(n + 1) * 512], os_)

    nc.compile()
    return nc


def _prep_inputs(x, W_qkv, W_proj):
    """Host-side shard + layout prep. Returns per-core input maps."""
    bf = ml_dtypes.bfloat16
    mtri = np.triu(np.ones((128, 128), dtype=np.float32)).astype(bf)
    mtri2 = np.concatenate(
        [np.zeros((128, 128), dtype=np.float32),
         np.triu(np.ones((128, 128), dtype=np.float32))], axis=1).astype(bf)
    in_maps = []
    for core in range(NCORES):
        b, g = core // 4, core % 4
        heads = range(4 * g, 4 * g + 4)
        # W_qkv rows: q = h*64.., k = C + h*64.., v = 2C + h*64..
        q_rows = np.concatenate([np.arange(h * HS, (h + 1) * HS) for h in heads])
        wqk = np.concatenate(
            [W_qkv[q_rows, :].T, W_qkv[C + q_rows, :].T], axis=1)  # [C, 512]
        wv = W_qkv[2 * C + q_rows, :].T                            # [C, 256]
        wp = W_proj[:, q_rows].T                                   # [256, C]
        in_maps.append({
            "xT": np.ascontiguousarray(x[b].T).astype(bf),
            "wqk": np.ascontiguousarray(wqk).astype(bf),
            "wv": np.ascontiguousarray(wv).astype(bf),
            "wp": np.ascontiguousarray(wp).astype(bf),
            "mtri": mtri,
            "mtri2": mtri2,
        })
    return in_maps


def _run(in_maps, trace=False, trace_cores=None):
    if "nc" not in _CACHE:
        _CACHE["nc"] = _build()
    return run_bass_kernel_spmd(
        _CACHE["nc"], in_maps, core_ids=list(range(NCORES)),
        trace=trace, trace_cores=trace_cores,
    )


def kernel(x, W_qkv, W_proj, b_proj):
    x = np.asarray(x, dtype=np.float32)
    W_qkv = np.asarray(W_qkv, dtype=np.float32)
    W_proj = np.asarray(W_proj, dtype=np.float32)
    b_proj = np.asarray(b_proj, dtype=np.float32)

    res = _run(_prep_inputs(x, W_qkv, W_proj)).results
    out = np.zeros((B, T, C), dtype=np.float64)
    for core in range(NCORES):
        out[core // 4] += np.asarray(res[core]["out"], dtype=np.float64)
    out += b_proj.astype(np.float64)
    return out.astype(np.float32)


# revision 22
# speedup vs baseline: 1.0439x; 1.0439x over previous
"""Fused sp2norm-MHA kernel for Trainium2, 8 NeuronCores.

Model (per reference):
    qkv = x @ W_qkv.T ; split heads (H=16, hs=64)
    s = (q @ k.T) / sqrt(hs);  w = softplus(s) causal-masked
    out_h = (w @ v) / ||w||_row ;  out = concat(out_h) @ W_proj.T + b_proj

Sharding: core c = (b, g) with b = c // 4 (batch), g = c % 4 (head group of 4).
Each core computes its batch's QKV for its 4 heads, the attention, and a
partial projection over its 256 feature channels. The host sums the 4 partial
projections per batch and adds the bias (the unshard step).

Schedule: single fully-interleaved phase. The attention pipeline (scores ->
softplus(Exp,Ln) -> mask/square -> out/norm matmuls) is ACT-bound (~2.05us
per j-block vs ~1.4us of PE work), so the QKV projection matmuls and the
output projection are DRIP-FED between attention blocks: the PE never idles
long enough for the HAM clock gate to re-throttle it to 1.2 GHz (which cost
the phase-separated version ~90us), and the QKV/proj phases hide entirely
inside the attention's ACT time.

PSUM (8 banks): scores [128,2048] x1 (4) | out.T [128,512] x2 (2) |
norm [128,512] x1 (1) | misc qkv/proj/bcast [128,512] x1 (1).

TWO j-blocks are batched per softplus ACT instruction (the ~293ns fixed
cost per ACTIVATE was ~47us over 160 instrs; batching halves it). In a
batch the narrower diagonal block is computed at the wider block's width;
the extra leading columns fall in the causally-masked region and are
zeroed by a widened [0|triu] mask, so they add exact zeros to the out/norm
accumulations.

The emission is software-pipelined: batch k's out/norm matmuls are emitted
AFTER batch k+1's score matmuls, so the in-order PE queue never stalls on
the ACT->DVE chain of the current batch.

The norm rows for the two head pairs share one PSUM bank (pair0 at
partitions 0/64, pair1 at 32/96) so one Ln+Exp epilogue pair serves all
4 heads of an i-chunk.
"""

import numpy as np
import ml_dtypes
from collections import deque

import concourse.bacc as bacc
import concourse.tile as tile
import concourse.mybir as mybir
from concourse.bass_utils import run_bass_kernel_spmd

# The act-table-set chooser assigns each activation the FIRST set containing
# its function; with the default ordering Exp -> exp_and_others and
# Ln -> natural_log, so alternating Exp/Ln thrashes ACT_TABLE_LOAD (~1.3us
# each, >100 loads). Reorder so the combined Exp+Ln set is preferred.
_orig_get_tables = bacc.get_activation_tables


def _tables_ln_exp_first(arch):
    t = _orig_get_tables(arch)
    key = "natural_log_exp_and_others"
    if key not in t:
        return t
    exp = mybir.ActivationFunctionType.Exp
    ln = mybir.ActivationFunctionType.Ln
    out = {}
    for k, fns in t.items():
        out[k] = fns if k == key else (set(fns) - {exp, ln})
    return out


bacc.get_activation_tables = _tables_ln_exp_first

dt = mybir.dt
F32, F16, BF16 = dt.float32, dt.float16, dt.bfloat16
AF = mybir.ActivationFunctionType

B, T, C, H, HS = 2, 2048, 1024, 16, 64
NCORES = 8
SCALE = 1.0 / np.sqrt(HS)

_CACHE = {}


def _build():
    nc = bacc.Bacc(None, target_bir_lowering=False)

    xT = nc.dram_tensor("xT", [C, T], BF16, kind="ExternalInput")
    wqk = nc.dram_tensor("wqk", [C, 512], BF16, kind="ExternalInput")
    wv = nc.dram_tensor("wv", [C, 256], BF16, kind="ExternalInput")
    wp = nc.dram_tensor("wp", [256, C], BF16, kind="ExternalInput")
    mtri = nc.dram_tensor("mtri", [128, 128], BF16, kind="ExternalInput")
    mtri2 = nc.dram_tensor("mtri2", [128, 256], BF16, kind="ExternalInput")
    out = nc.dram_tensor("out", [T, C], F32, kind="ExternalOutput")

    with tile.TileContext(nc) as tc:
        with (
            tc.tile_pool(name="cst", bufs=1) as cst,
            tc.tile_pool(name="data", bufs=1) as data,
            tc.tile_pool(name="ps", bufs=1, space="PSUM") as ps,
            tc.tile_pool(name="we", bufs=2) as we,
            tc.tile_pool(name="epi", bufs=2) as epi,
            tc.tile_pool(name="outp", bufs=3) as outp,
        ):
            # ---- constants / weights ----
            # Input DMAs are spread across the three DMA-capable engines
            # (sync, scalar, gpsimd): DMA_DIRECT2D occupies its issuing
            # engine for ~0.6us per 128KB, so one queue serializes the
            # ~6MB of inputs into a ~26us lead-in. Three queues + cb/t-quarter
            # splitting get the first QKV group's inputs in ~3us.
            mtri_sb = cst.tile([128, 128], BF16)
            nc.gpsimd.dma_start(mtri_sb, mtri[:])
            mtri2_sb = cst.tile([128, 256], BF16)
            nc.gpsimd.dma_start(mtri2_sb, mtri2[:])
            ones_n = cst.tile([128, 1], BF16)
            nc.vector.memset(ones_n, 1.0)
            ones_b = cst.tile([128, 64], BF16)
            nc.vector.memset(ones_b, 1.0)

            wqk_sb = cst.tile([128, 8, 512], BF16)
            wv_sb = cst.tile([128, 8, 256], BF16)
            xT_sb = data.tile([128, 8, 2048], BF16)
            wqk_r = wqk[:].rearrange("(po pi) j -> pi po j", pi=128)
            wv_r = wv[:].rearrange("(po pi) j -> pi po j", pi=128)
            xT_r = xT[:].rearrange("(po pi) t -> pi po t", pi=128)
            for cb in range(8):
                nc.scalar.dma_start(wqk_sb[:, cb, :], wqk_r[:, cb, :])
            # first QKV groups only need t[0:512]: stream xT by t-quarter
            for cb in range(8):
                eng = nc.sync if cb % 2 == 0 else nc.gpsimd
                eng.dma_start(xT_sb[:, cb, 0:512], xT_r[:, cb, 0:512])
            for cb in range(8):
                nc.gpsimd.dma_start(wv_sb[:, cb, :], wv_r[:, cb, :])
            for tq in range(1, 4):
                for cb in range(8):
                    eng = nc.sync if cb % 2 == 0 else nc.gpsimd
                    eng.dma_start(xT_sb[:, cb, tq * 512:(tq + 1) * 512],
                                  xT_r[:, cb, tq * 512:(tq + 1) * 512])
            wp_sb = cst.tile([128, 2, 1024], BF16)
            nc.scalar.dma_start(wp_sb, wp[:].rearrange("(po pi) e -> pi po e", pi=128))

            # qkT: block 0,1 = q head-pairs; block 2,3 = k head-pairs.
            # Partition rows (h%2)*64..+64 inside each block = one head.
            qkT = data.tile([128, 4, 2048], BF16)
            v_sb = data.tile([128, 16, 256], BF16)
            yT = data.tile([128, 2, 2048], BF16)

            # ---------- drip-fed work items (QKV groups, proj chunks) ----------
            # Each item is ~0.9-1.8us of PE work, emitted between attention
            # blocks so the in-order PE queue always has ready work.
            def emit_qk_group(jb, tcc):
                # qkT[:, jb, tcc*512:+512] = wqk_blk.T @ xT_chunk
                pq = ps.tile([128, 512], F32, tag="m", bufs=1)
                for cb in range(8):
                    nc.tensor.matmul(
                        pq,
                        wqk_sb[:, cb, jb * 128:(jb + 1) * 128],
                        xT_sb[:, cb, tcc * 512:(tcc + 1) * 512],
                        start=(cb == 0), stop=(cb == 7),
                    )
                nc.vector.tensor_copy(qkT[:, jb, tcc * 512:(tcc + 1) * 512], pq)

            def emit_v_group(tb):
                pv = ps.tile([128, 512], F32, tag="m", bufs=1)
                for cb in range(8):
                    nc.tensor.matmul(
                        pv[:, 0:256],
                        xT_sb[:, cb, tb * 128:(tb + 1) * 128],
                        wv_sb[:, cb, :],
                        start=(cb == 0), stop=(cb == 7),
                    )
                nc.vector.tensor_copy(v_sb[:, tb, :], pv[:, 0:256])

            def emit_proj(tcc, nk, sbank=False):
                if sbank:
                    pp_s = ps.tile([128, 2048], F32, tag="s", bufs=1, name="pp_s")
                    pp = pp_s[:, 0:512]
                else:
                    pp = ps.tile([128, 512], F32, tag="m", bufs=1)
                for kb in range(2):
                    nc.tensor.matmul(
                        pp,
                        yT[:, kb, tcc * 128:(tcc + 1) * 128],
                        wp_sb[:, kb, nk * 512:(nk + 1) * 512],
                        start=(kb == 0), stop=(kb == 1),
                    )
                os_ = outp.tile([128, 512], F32, tag="os")
                nc.vector.tensor_copy(os_, pp)
                eng = nc.scalar if (tcc + nk) % 2 else nc.sync
                eng.dma_start(
                    out[tcc * 128:(tcc + 1) * 128, nk * 512:(nk + 1) * 512], os_)

            qkv_q = deque()
            proj_q = deque()

            fill_ok = [False]

            def drip(k):
                for _ in range(k):
                    if qkv_q:
                        qkv_q.popleft()()
                    elif proj_q:
                        t, n = proj_q.popleft()
                        emit_proj(t, n)
                    elif fill_ok[0]:
                        # dummy matmuls keep the HAM activity window busy in
                        # the PE-sparse late chunks (else cold 1.2 GHz tail)
                        fp = ps.tile([128, 512], F32, tag="m", bufs=1,
                                     name="fill")
                        for _ in range(3):
                            nc.tensor.matmul(fp[0:64, :], ones_b, wrm,
                                             start=True, stop=True)
                        return
                    else:
                        return

            # ---------- attention block ----------
            PN_ROW = {0: (0, 64), 1: (32, 96)}   # pn partition rows per pair

            # pending[0] holds the deferred out/norm emission of the previous
            # block: it is emitted AFTER the next block's score matmuls so
            # those reach the in-order PE queue first and fill the Exp shadow.
            pending = [None]

            def flush_pending():
                if pending[0] is not None:
                    pending[0]()
                    pending[0] = None

            def emit_att_batch(hp, ic, bb, po, pn):
                # batch = j-blocks (2bb, 2bb+1); njb = 4ic+4 is always even.
                njb = 4 * ic + 4
                jb0, jb1 = 2 * bb, 2 * bb + 1
                m0 = jb0 - 4 * ic
                # both blocks computed at the wider block's width N
                N = 512 if m0 < 0 else 512 - 128 * m0
                diag = m0 >= 0
                ioff = ic * 512 + (512 - N)
                qblk, kblk = hp, 2 + hp
                ps_ = ps.tile([128, 2048], F32, tag="s", bufs=1)
                # scores (transposed): K=64, heads row-packed; segments:
                # [jb0/headA | jb0/headB | jb1/headA | jb1/headB] at 512 pitch
                for si, (jb, p0, p1) in enumerate(
                        ((jb0, 0, 64), (jb0, 64, 128),
                         (jb1, 0, 64), (jb1, 64, 128))):
                    nc.tensor.matmul(
                        ps_[:, si * 512:si * 512 + N],
                        qkT[p0:p1, kblk, jb * 128:(jb + 1) * 128],
                        qkT[p0:p1, qblk, ioff:ioff + N],
                        start=True, stop=True,
                    )
                # softplus = Ln(Exp(s/8) + 1), fp16 intermediate, one ACT
                # instruction per pass over all 4 segments.
                e = we.tile([128, 2048], F16, tag="e")
                w = we.tile([128, 2048], BF16, tag="w", bufs=3)
                w2 = we.tile([128, 2048], BF16, tag="w2", bufs=3)
                if N == 512:
                    nc.scalar.activation(e, ps_, AF.Exp, scale=SCALE)
                    nc.scalar.activation(w, e, AF.Ln, bias=1.0)
                else:
                    ps4 = ps_.rearrange("p (b n) -> p b n", b=4)[:, :, 0:N]
                    e4 = e.rearrange("p (b n) -> p b n", b=4)[:, :, 0:N]
                    w4 = w.rearrange("p (b n) -> p b n", b=4)[:, :, 0:N]
                    nc.scalar.activation(e4, ps4, AF.Exp, scale=SCALE)
                    nc.scalar.activation(w4, e4, AF.Ln, bias=1.0)
                if diag:
                    # jb0: diagonal sub-block = leading 128 cols of each head
                    # segment; jb1: leading 128 are fully masked (they were
                    # computed only to keep the batch width uniform) and the
                    # next 128 are its diagonal -> [0|triu] 256-wide mask.
                    nc.vector.tensor_mul(w[:, 0:128], w[:, 0:128], mtri_sb)
                    nc.vector.tensor_mul(w[:, 512:640], w[:, 512:640], mtri_sb)
                    nc.vector.tensor_mul(w[:, 1024:1280], w[:, 1024:1280],
                                         mtri2_sb)
                    nc.vector.tensor_mul(w[:, 1536:1792], w[:, 1536:1792],
                                         mtri2_sb)
                if N == 512:
                    nc.vector.tensor_mul(w2, w, w)
                else:
                    w4 = w.rearrange("p (b n) -> p b n", b=4)[:, :, 0:N]
                    w24 = w2.rearrange("p (b n) -> p b n", b=4)[:, :, 0:N]
                    nc.vector.tensor_mul(w24, w4, w4)

                # out/norm of the PREVIOUS batch go to the PE queue now...
                flush_pending()
                drip(1)

                def out_norm(first=(bb == 0), last=(jb1 == njb - 1), jb0=jb0,
                             jb1=jb1, hp=hp, N=N, w=w, w2=w2, po=po, pn=pn):
                    hA, hB = 2 * hp, 2 * hp + 1
                    rA, rB = PN_ROW[hp]
                    for bi, jb in ((0, jb0), (1, jb1)):
                        st = first and bi == 0
                        sp = last and bi == 1
                        nc.tensor.matmul(
                            po[0:64, 512 - N:512],
                            v_sb[:, jb, hA * 64:hA * 64 + 64],
                            w[:, bi * 1024:bi * 1024 + N],
                            start=st, stop=sp, tile_position=(0, 0),
                        )
                        nc.tensor.matmul(
                            po[64:128, 512 - N:512],
                            v_sb[:, jb, hB * 64:hB * 64 + 64],
                            w[:, bi * 1024 + 512:bi * 1024 + 512 + N],
                            start=st, stop=sp, tile_position=(0, 64),
                        )
                        nc.tensor.matmul(
                            pn[rA:rA + 1, 512 - N:512], ones_n,
                            w2[:, bi * 1024:bi * 1024 + N],
                            start=st, stop=sp, tile_position=(0, rA),
                        )
                        nc.tensor.matmul(
                            pn[rB:rB + 1, 512 - N:512], ones_n,
                            w2[:, bi * 1024 + 512:bi * 1024 + 512 + N],
                            start=st, stop=sp, tile_position=(0, rB),
                        )

                # ...this batch's out/norm wait for the next batch's scores
                pending[0] = out_norm

            # ---------- main interleaved schedule ----------
            # PE warmup: ~3.5us of back-to-back tiny matmuls trips the HAM
            # activity window so the clock gate opens (1.2 -> 2.4 GHz) before
            # the first real QKV matmuls; they only depend on the ones memset.
            wrm = cst.tile([128, 512], BF16)
            nc.vector.memset(wrm, 0.0)
            # prereq: q-pair0/k-pair0 i-chunk 0, v block 0 -- all three
            # groups' matmuls interleaved into quarters of ONE score tile so
            # they chase the arriving DMA slices together (multiple writers
            # to a tile don't serialize; only read-then-write does).
            pre = ps.tile([128, 2048], F32, tag="s", bufs=1, name="pre")
            for cb in range(8):
                st, sp = cb == 0, cb == 7
                nc.tensor.matmul(pre[:, 0:512], wqk_sb[:, cb, 0:128],
                                 xT_sb[:, cb, 0:512], start=st, stop=sp)
                nc.tensor.matmul(pre[:, 512:1024], wqk_sb[:, cb, 256:384],
                                 xT_sb[:, cb, 0:512], start=st, stop=sp)
                nc.tensor.matmul(pre[:, 1024:1280], xT_sb[:, cb, 0:128],
                                 wv_sb[:, cb, :], start=st, stop=sp)
            nc.vector.tensor_copy(qkT[:, 0, 0:512], pre[:, 0:512])
            nc.vector.tensor_copy(qkT[:, 2, 0:512], pre[:, 512:1024])
            nc.vector.tensor_copy(v_sb[:, 0, :], pre[:, 1024:1280])
            # Release QKV groups just-in-time (keyed by (ic, hp) attention
            # segment) so PE drip work stays dense through the LATE i-chunks,
            # where the attention itself leaves the PE half idle -- otherwise
            # the HAM clock gate re-throttles the PE to 1.2 GHz there.
            RELEASES = {
                (0, 0): [("v", 1), ("v", 2), ("v", 3),
                         ("qk", 1, 0), ("qk", 3, 0)],
                (0, 1): [("qk", 0, 1), ("qk", 2, 1),
                         ("v", 4), ("v", 5), ("v", 6), ("v", 7)],
                (1, 0): [("qk", 1, 1), ("qk", 3, 1)],
                (1, 1): [("qk", 0, 2), ("qk", 2, 2),
                         ("v", 8), ("v", 9), ("v", 10), ("v", 11)],
                (2, 0): [("qk", 1, 2), ("qk", 3, 2)],
                (2, 1): [("qk", 0, 3), ("qk", 2, 3)],
                (3, 0): [("v", 12), ("v", 13), ("v", 14), ("v", 15),
                         ("qk", 1, 3), ("qk", 3, 3)],
            }

            def release(ic, hp):
                for it in RELEASES.get((ic, hp), []):
                    if it[0] == "v":
                        qkv_q.append(lambda tb=it[1]: emit_v_group(tb))
                    else:
                        qkv_q.append(
                            lambda jb=it[1], tcc=it[2]: emit_qk_group(jb, tcc))

            # The chunk epilogue is ALSO deferred: it is emitted after the
            # NEXT chunk's first batch so its ACT ops (Ln/Exp on the norms)
            # slot in behind that batch's softplus instead of stalling the
            # ACT queue at the chunk boundary.
            pending_epi = [None]

            def flush_epi():
                if pending_epi[0] is not None:
                    pending_epi[0]()
                    pending_epi[0] = None

            def make_epi(ic, pn, po_hp):
                def epi_fn():
                    # ---- chunk epilogue: y = out.T * rsqrt(norm2), both
                    # pairs. rsqrt = Exp(-0.5*Ln(x)): stays in the Exp/Ln set.
                    nrm = epi.tile([128, 512], F32, tag="nrm")
                    nc.scalar.activation(nrm, pn, AF.Ln)
                    rs = epi.tile([128, 512], BF16, tag="rs")
                    nc.scalar.activation(rs, nrm, AF.Exp, scale=-0.5)
                    # keep the PE fed through the epilogue's ACT/DVE window
                    drip(2)
                    for hp in range(2):
                        rA, rB = PN_ROW[hp]
                        pb = ps.tile([128, 512], F32, tag="m", bufs=1)
                        nc.tensor.matmul(pb[0:64, :], ones_b[rA:rA + 1, :],
                                         rs[rA:rA + 1, :], start=True,
                                         stop=True, tile_position=(rA, 0))
                        nc.tensor.matmul(pb[64:128, :], ones_b[rB:rB + 1, :],
                                         rs[rB:rB + 1, :], start=True,
                                         stop=True, tile_position=(rB, 64))
                        rb = epi.tile([128, 512], F32, tag="rb")
                        nc.vector.tensor_copy(rb, pb)
                        nc.vector.tensor_mul(
                            yT[:, hp, ic * 512:(ic + 1) * 512], po_hp[hp], rb)
                    # projection for this i-chunk now has all its inputs
                    for tcc in range(ic * 4, ic * 4 + 4):
                        for nk in range(2):
                            proj_q.append((tcc, nk))
                return epi_fn

            for ic in range(4):
                fill_ok[0] = ic >= 2
                pn = ps.tile([128, 512], F32, tag="n", bufs=1)
                po_hp = {}
                for hp in range(2):
                    release(ic, hp)
                    po = ps.tile([128, 512], F32, tag="o", bufs=2)
                    po_hp[hp] = po
                    for bb in range(2 * ic + 2):
                        emit_att_batch(hp, ic, bb, po, pn)
                        if hp == 0 and bb == 0:
                            flush_epi()   # previous chunk's epilogue
                        if ic == 0:
                            drip(2)   # extra drip early: QKV is front-loaded
                flush_pending()
                pending_epi[0] = make_epi(ic, pn, po_hp)
            flush_epi()

            # drain whatever is left (tail projection chunks): the four
            # score banks are free now -- run 4 chunks in parallel through
            # the quarters of one [128,2048] allocation, DMAs spread over
            # 4 engine queues.
            while qkv_q:
                qkv_q.popleft()()
            tail = list(proj_q)
            proj_q.clear()
            engs = [nc.sync, nc.scalar, nc.gpsimd]
            TROT = [("m", 1), ("o", 2), ("o", 2), ("n", 1)]
            for qi, (t, n) in enumerate(tail):
                tg, bfs = TROT[qi % 4]
                pp = ps.tile([128, 512], F32, tag=tg, bufs=bfs,
                             name=f"tp{qi}")
                for kb in range(2):
                    nc.tensor.matmul(
                        pp,
                        yT[:, kb, t * 128:(t + 1) * 128],
                        wp_sb[:, kb, n * 512:(n + 1) * 512],
                        start=(kb == 0), stop=(kb == 1),
                    )
                os_ = outp.tile([128, 512], F32, tag="ost", bufs=4)
                nc.vector.tensor_copy(os_, pp)
                engs[qi % 3].dma_start(
                    out[t * 128:(t + 1) * 128, n * 512:(n + 1) * 512], os_)

    nc.compile()
    return nc


def _prep_inputs(x, W_qkv, W_proj):
    """Host-side shard + layout prep. Returns per-core input maps."""
    bf = ml_dtypes.bfloat16
    mtri = np.triu(np.ones((128, 128), dtype=np.float32)).astype(bf)
    mtri2 = np.concatenate(
        [np.zeros((128, 128), dtype=np.float32),
         np.triu(np.ones((128, 128), dtype=np.float32))], axis=1).astype(bf)
    in_maps = []
    for core in range(NCORES):
        b, g = core // 4, core % 4
        heads = range(4 * g, 4 * g + 4)
        # W_qkv rows: q = h*64.., k = C + h*64.., v = 2C + h*64..
        q_rows = np.concatenate([np.arange(h * HS, (h + 1) * HS) for h in heads])
        wqk = np.concatenate(
            [W_qkv[q_rows, :].T, W_qkv[C + q_rows, :].T], axis=1)  # [C, 512]
        wv = W_qkv[2 * C + q_rows, :].T                            # [C, 256]
        wp = W_proj[:, q_rows].T                                   # [256, C]
        in_maps.append({
            "xT": np.ascontiguousarray(x[b].T).astype(bf),
            "wqk": np.ascontiguousarray(wqk).astype(bf),
            "wv": np.ascontiguousarray(wv).astype(bf),
            "wp": np.ascontiguousarray(wp).astype(bf),
            "mtri": mtri,
            "mtri2": mtri2,
        })
    return in_maps


def _run(in_maps, trace=False, trace_cores=None):
    if "nc" not in _CACHE:
        _CACHE["nc"] = _build()
    return run_bass_kernel_spmd(
        _CACHE["nc"], in_maps, core_ids=list(range(NCORES)),
        trace=trace, trace_cores=trace_cores,
    )


def kernel(x, W_qkv, W_proj, b_proj):
    x = np.asarray(x, dtype=np.float32)
    W_qkv = np.asarray(W_qkv, dtype=np.float32)
    W_proj = np.asarray(W_proj, dtype=np.float32)
    b_proj = np.asarray(b_proj, dtype=np.float32)

    res = _run(_prep_inputs(x, W_qkv, W_proj)).results
    out = np.zeros((B, T, C), dtype=np.float64)
    for core in range(NCORES):
        out[core // 4] += np.asarray(res[core]["out"], dtype=np.float64)
    out += b_proj.astype(np.float64)
    return out.astype(np.float32)
